# revision 59
# baseline (speedup 1.0000x reference)
"""Multi-head attention (B=2, L=2048, H=16, D=64) on 8 TRN2 NeuronCores.

Sharding: core = (batch b, head-group hg); 2 batches x 4 groups of 4 heads.
Per core, for its batch and its 4 heads (2 head-pairs m):
    Q^T/K^T = W^T x^T           (pair-d on partitions; head 2m at rows 0:64,
                                 head 2m+1 at rows 64:128)
    V       = x W_v             (j on partitions, + ones column for denom)
    S^T     = K^T.T Q^T         (j on partitions, i free)
    P'      = exp(S^T/8)        (un-normalized softmax numerator, bf16)
    O'^T    = [V|1].T P'        (row 64 = softmax denominator)
    O^T     = O'[0:64] / O'[64]
    out^T  += Wo_rows^T O^T     (partial over head-group rows of Wo)
Host sums the 4 partials per batch, transposes, adds bo.

v2 schedule (from perfetto/NTFF analysis of the 286us baseline):
  - everything bf16 (same 0.414ns/row PE rate as f32r at N>=512, but half
    the DMA bytes; psum accumulation stays f32).
  - HW model: matmul ~= 13ns + N*0.414ns; LDWEIGHTS ~= K*1.2ns hidden
    under the previous matmul's streaming; exp [128,1024] ~= 1115ns + a
    ~114ns semaphore wait on the ACT queue -> attention is ACT-paced at
    ~2.4us per j-step, with ~0.5us/step of PE slack.
  - the baseline spent 85us on a serial DMA+projection head and 46us on a
    serial Wo+output tail; v2 streams projections/V/Wo-half-0 into the
    attention loop's PE slack and starts attention after only ~3 proj
    chunks (~13us).
  - normalize: DVE reciprocal straight on the [1,1024] denominator row,
    gpsimd partition_broadcast, DVE multiply; head-odd rows reach
    oT partitions 64:128 via a gpsimd SBUF->SBUF dma (engine lanes can't
    shift partitions).
"""

import sys

try:
    import concourse.bass as bass  # noqa: F401
except ImportError:  # pragma: no cover - path fallback
    sys.path.insert(0, "/opt/trn_rl_repo")

import numpy as np
import ml_dtypes
import concourse.bass as bass
import concourse.mybir as mybir
import concourse.tile as tile
from concourse import bacc
from concourse.bass_utils import run_bass_kernel_spmd

F32 = mybir.dt.float32
F32R = mybir.dt.float32r
BF16 = mybir.dt.bfloat16
AF = mybir.ActivationFunctionType

B = 2
L = 2048          # sequence length
C = 1024          # model dim
H_LOC = 4         # heads per core
D = 64            # head dim
HD = H_LOC * D    # 256 = local head-group width
KT = C // 128     # 8 k-tiles over the model dim
SCALE2 = float(D) ** -0.5  # 1/8, applied once inside exp

_cache = {}


def _build():
    nc = bacc.Bacc("TRN2", target_bir_lowering=False, debug=False, num_devices=8)

    # weights arrive host-pre-transposed to partition-major [128, k, c]
    # layout so their DMA is one contiguous run per partition (the natural
    # [(k p), c] rearrange loads 512B-strided gathers)
    xT = nc.declare_dram_parameter("xT", [C, L], BF16, isOutput=False)
    wq = nc.declare_dram_parameter("wq", [128, KT * HD], BF16, isOutput=False)
    wk = nc.declare_dram_parameter("wk", [128, KT * HD], BF16, isOutput=False)
    wv = nc.declare_dram_parameter("wv", [128, KT * HD], BF16, isOutput=False)
    wo = nc.declare_dram_parameter("wo", [128, 2 * C], BF16, isOutput=False)
    outT = nc.declare_dram_parameter("outT", [C, L], BF16, isOutput=True)

    with tile.TileContext(nc) as tc:
        with tc.tile_pool(name="sb", bufs=1) as sb, \
             tc.tile_pool(name="ps", bufs=2, space="PSUM") as ps, \
             tc.tile_pool(name="po", bufs=2, space="PSUM") as po:

            # ---- input DMA (issue order == availability order) -----------
            wq_sb = sb.tile([128, KT, HD], BF16, tag="wq")
            wk_sb = sb.tile([128, KT, HD], BF16, tag="wk")
            wv_sb = sb.tile([128, KT, HD], BF16, tag="wv")
            wo_sb = sb.tile([128, 2, C], BF16, tag="wo")
            xT_sb = sb.tile([128, KT, L], BF16, tag="xT")
            # few DMA issues (each dma_start costs ~650ns on the in-order
            # Sync queue), granular enough that the first proj chunks can
            # start before the whole xT half has landed
            xTr = xT.rearrange("(k p) l -> p k l", p=128)
            nc.sync.dma_start(xT_sb[:, 0:2, 0:1024], xTr[:, 0:2, 0:1024])
            nc.sync.dma_start(wq_sb[:, :, :], wq.rearrange("p (k c) -> p k c", k=KT))
            nc.sync.dma_start(wk_sb[:, :, :], wk.rearrange("p (k c) -> p k c", k=KT))
            for k in range(2, KT, 2):
                nc.sync.dma_start(xT_sb[:, k:k + 2, 0:1024], xTr[:, k:k + 2, 0:1024])
            nc.sync.dma_start(wv_sb[:, :, :], wv.rearrange("p (k c) -> p k c", k=KT))
            for k in range(0, KT, 4):
                nc.sync.dma_start(xT_sb[:, k:k + 4, 1024:2048],
                                  xTr[:, k:k + 4, 1024:2048])
            nc.sync.dma_start(wo_sb[:, :, :], wo.rearrange("p (k c) -> p k c", k=2))

            # ---- persistent SBUF ----------------------------------------
            qT_sb = sb.tile([128, 2, L], BF16, tag="qT")
            kT_sb = sb.tile([128, 2, L], BF16, tag="kT")
            # per-(i-half, pair) attention output tiles: avoids tile-granular
            # false deps between Wo reads and normalize writes, and keeps the
            # normalize-mult dest AP flat (a [0:64, m, :] view runs 2x slower
            # on DVE than a flat [64, 1024] write)
            oT_sbs = [[sb.tile([128, 1024], BF16, tag=f"oT{ih}{m}",
                               name=f"oT{ih}{m}")
                       for m in range(2)]
                      for ih in range(2)]
            v_sb = sb.tile([128, 16, H_LOC, D + 1], BF16, tag="v")
            nc.vector.memset(v_sb[:, :, :, D:D + 1], 1.0)

            es_pool = tc.alloc_tile_pool(name="es_pool", bufs=8)
            np_pool = tc.alloc_tile_pool(name="np_pool", bufs=4)
            nd_pool = tc.alloc_tile_pool(name="nd_pool", bufs=4)
            st_pool = tc.alloc_tile_pool(name="st_pool", bufs=2)
            ost_pool = tc.alloc_tile_pool(name="ost_pool", bufs=4)

            # ---- emission helpers ---------------------------------------
            def emit_pchunk(w_sb, t_sb, m, n, on_act=False):
                # q/k projection chunk: [128 pair-d, 512 i|j] of qT/kT.
                # on_act: do the psum->bf16 copy on the Scalar engine — in
                # the PE-bound early units ACT has idle slots and every DVE
                # op is coupling fodder for coalesced j-loop waits.
                p = ps.tile([128, 1024], F32, tag="s", name="proj")
                acc = p[:, 0:512]
                for k in range(KT):
                    nc.tensor.matmul(
                        acc,
                        w_sb[:, k, m * 128:(m + 1) * 128],
                        xT_sb[:, k, n * 512:(n + 1) * 512],
                        start=(k == 0), stop=(k == KT - 1),
                    )
                dst = t_sb[:, m, n * 512:(n + 1) * 512]
                if on_act:
                    nc.scalar.copy(dst, acc)
                else:
                    with nc.allow_low_precision(reason="bf16 matmul input"):
                        nc.vector.tensor_copy(dst, acc)

            def emit_vchunk(it, on_act=False):
                # V projection for one j-tile: [128 j, 4 h, 64 d]
                p = ps.tile([128, 1024], F32, tag="s", name="vproj")
                acc = p[:, 0:HD]
                for k in range(KT):
                    nc.tensor.matmul(
                        acc,
                        xT_sb[:, k, it * 128:(it + 1) * 128],
                        wv_sb[:, k, :],
                        start=(k == 0), stop=(k == KT - 1),
                    )
                dst = v_sb[:, it, :, 0:D]
                src = acc.rearrange("p (h d) -> p h d", h=H_LOC)
                if on_act:
                    nc.scalar.copy(dst, src)
                else:
                    with nc.allow_low_precision(reason="bf16 matmul input"):
                        nc.vector.tensor_copy(dst, src)

            def emit_wo_chunk(ih, ct, cast_act=False):
                # [128 c-rows, 1024 i] output chunk; accumulate both pairs
                i0 = ih * 1024
                acc = ps.tile([128, 1024], F32, tag="s", name="wo_ps")
                for kk in range(2):
                    for n in range(2):
                        nc.tensor.matmul(
                            acc[:, n * 512:(n + 1) * 512],
                            wo_sb[:, kk, ct * 128:(ct + 1) * 128],
                            oT_sbs[ih][kk][:, n * 512:(n + 1) * 512],
                            start=(kk == 0), stop=(kk == 1),
                        )
                ost = ost_pool.tile([128, 1024], BF16, tag="ost", name="ost")
                if cast_act:
                    nc.scalar.copy(ost[:], acc[:])
                else:
                    with nc.allow_low_precision(reason="bf16 output"):
                        nc.vector.tensor_copy(ost[:], acc[:])
                nc.sync.dma_start(
                    outT[ct * 128:(ct + 1) * 128, i0:i0 + 1024], ost[:])

            def wo_mm(acc, ih, ct, kk, start, stop):
                for n in range(2):
                    nc.tensor.matmul(
                        acc[:, n * 512:(n + 1) * 512],
                        wo_sb[:, kk, ct * 128:(ct + 1) * 128],
                        oT_sbs[ih][kk][:, n * 512:(n + 1) * 512],
                        start=start, stop=stop,
                    )

            # deferred normalization state: (m, i0, o_cps)
            def emit_ocp_copy(m, i0, o_h, hl1_act=False):
                # hl1_act: in the tail ACT is idle — run the two copies on
                # different engines so they overlap
                o_cps = []
                for hl in range(2):
                    o_cp = np_pool.tile([65, 1024], F32, tag="o_cp",
                                        name=f"o_cp{hl}")
                    if hl and hl1_act:
                        nc.scalar.copy(o_cp[:], o_h[hl][:])
                    else:
                        nc.vector.tensor_copy(o_cp[:], o_h[hl][:])
                    o_cps.append(o_cp)
                return (m, i0, o_cps)

            def emit_norm_recip(pend):
                # reciprocal of the [1,1024] denominator row at full DVE
                # width via a [128,8] dma-reshape (single-partition DVE ops
                # run ~6.5us and clog the queue); everything except the
                # reciprocal itself runs on the otherwise-idle Pool engine
                # so DVE head-of-line waits can't stall the j-loop
                m, i0, o_cps = pend
                dsqs, outs = [], []
                for hl in range(2):
                    dsq = nd_pool.tile([128, 8], F32, tag=f"dsq{hl}")
                    nc.sync.dma_start(dsq[:], o_cps[hl][64:65, :])
                    dsqs.append(dsq)
                for hl in range(2):
                    nc.vector.reciprocal(dsqs[hl][:], dsqs[hl][:])
                for hl in range(2):
                    dinv = nd_pool.tile([1, 1024], F32, tag=f"dinv{hl}")
                    nc.sync.dma_start(dinv[:], dsqs[hl][:])
                    rep = nd_pool.tile([64, 1024], F32, tag=f"rep{hl}")
                    nc.gpsimd.partition_broadcast(rep[:], dinv[:])
                    outs.append(rep)
                return outs

            def emit_norm_mult(pend, reps, hl, via_stage=False):
                # hl0 writing the 128-partition oT tile directly costs ~2.5us
                # on DVE vs 1.2us into a 64-partition stage; in the tail the
                # chain is latency-critical so stage+HWDGE-dma both halves
                m, i0, o_cps = pend
                ih = i0 // 1024
                with nc.allow_low_precision(reason="bf16 attention output"):
                    if hl == 0 and not via_stage:
                        nc.vector.tensor_mul(
                            oT_sbs[ih][m][0:64, :],
                            o_cps[hl][0:64, :], reps[hl][:])
                    else:
                        stage = st_pool.tile([64, 1024], BF16, tag="stage")
                        nc.vector.tensor_mul(
                            stage[:], o_cps[hl][0:64, :], reps[hl][:])
                        # HWDGE, not gpsimd: PE reads of oT would otherwise
                        # pick up a coalesced full-gpsimd-queue drain dep
                        nc.sync.dma_start(
                            oT_sbs[ih][m][hl * 64:(hl + 1) * 64, :], stage[:])

            # ---- head: q/k (m=0, i|j 0:1024) as four k-interleaved chunks
            # across 4 psum accumulators (2 ps + 2 po slots, all free before
            # attention) so they complete right behind the xT DMA stream
            # instead of serializing chunk-by-chunk at mid p-state
            hq0 = ps.tile([128, 1024], F32, tag="s", name="hq0")
            hq1 = ps.tile([128, 1024], F32, tag="s", name="hq1")
            hk0 = po.tile([128, 1024], F32, tag="o", name="hk0")
            hk1 = po.tile([128, 1024], F32, tag="o", name="hk1")
            for k in range(KT):
                for acc, w_sb, n in ((hq0, wq_sb, 0), (hq1, wq_sb, 1),
                                     (hk0, wk_sb, 0), (hk1, wk_sb, 1)):
                    nc.tensor.matmul(
                        acc[:, 0:512],
                        w_sb[:, k, 0:128],
                        xT_sb[:, k, n * 512:(n + 1) * 512],
                        start=(k == 0), stop=(k == KT - 1),
                    )
            for acc, t_sb, n in ((hq0, qT_sb, 0), (hq1, qT_sb, 1),
                                 (hk0, kT_sb, 0), (hk1, kT_sb, 1)):
                nc.scalar.copy(t_sb[:, 0, n * 512:(n + 1) * 512],
                               acc[:, 0:512])
            emit_vchunk(0, True)
            emit_vchunk(1, True)

            # extra PE/normalize work interleaved into the j-loops.
            # item: (ui, j, thunk) emitted right before step j of unit ui.
            extras = []

            def at(ui, j, fn, *a):
                extras.append((ui, j, fn, a))

            # unit 0 carries: rest of V and its own late K, unit 1's Q/K
            # (all psum->sbuf copies on ACT: it idles in this PE-bound span)
            for t in range(2, 16):                          # v jt 2..15 JIT
                at(0, t - 1, emit_vchunk, t, True)
            at(0, 5, emit_pchunk, wk_sb, kT_sb, 0, 2, True)   # u0 j8..11
            at(0, 9, emit_pchunk, wk_sb, kT_sb, 0, 3, True)   # u0 j12..15
            at(0, 11, emit_pchunk, wq_sb, qT_sb, 1, 0, True)  # u1 i 0..511
            at(0, 13, emit_pchunk, wq_sb, qT_sb, 1, 1, True)  # u1 i 512..1023
            at(0, 15, emit_pchunk, wk_sb, kT_sb, 1, 0, True)  # u1 j0..3
            # unit 1 carries: rest of its own K, unit 2's Q
            at(1, 0, emit_pchunk, wk_sb, kT_sb, 1, 1, True)   # u1 j4..7
            at(1, 4, emit_pchunk, wk_sb, kT_sb, 1, 2, True)   # u1 j8..11
            at(1, 6, emit_pchunk, wk_sb, kT_sb, 1, 3, True)   # u1 j12..15
            at(1, 9, emit_pchunk, wq_sb, qT_sb, 0, 2)         # u2 i 0..511
            at(1, 11, emit_pchunk, wq_sb, qT_sb, 0, 3)        # u2 i 512..1023
            # unit 2 carries: unit 3's Q
            at(2, 5, emit_pchunk, wq_sb, qT_sb, 1, 2)       # u3
            at(2, 7, emit_pchunk, wq_sb, qT_sb, 1, 3)       # u3
            # unit 3 carries all of Wo ih=0: norm(u1) finishes emission at
            # u2 step 15, so emission here guarantees the oT0 read order
            for ct, st in enumerate((2, 4, 6, 8, 10, 12, 14, 15)):
                at(3, st, emit_wo_chunk, 0, ct)

            # ---- attention units ----------------------------------------
            units = [(ih, m) for ih in range(2) for m in range(2)]
            pending = None   # o_cp record awaiting normalize
            reps = None

            for ui, (ih, m) in enumerate(units):
                i0 = ih * 1024
                o_h = []
                for hl in range(2):
                    of = po.tile([128, 1024], F32, tag="o", name=f"o_ps{hl}")
                    o_h.append(of[0:65, :])
                es = [None, None]
                for j in range(16):
                    # S first so exp can fire ASAP; its psum slot was freed
                    # by exp(j-2)
                    s_list = []
                    for hl in range(2):
                        r0 = hl * 64
                        s_ps = ps.tile([128, 1024], F32, tag="s", name=f"s_ps{hl}")
                        for n in range(2):
                            nc.tensor.matmul(
                                s_ps[:, n * 512:(n + 1) * 512],
                                kT_sb[r0:r0 + 64, m, j * 128:(j + 1) * 128],
                                qT_sb[r0:r0 + 64, m,
                                      i0 + n * 512:i0 + (n + 1) * 512],
                                start=True, stop=True,
                            )
                        s_list.append(s_ps)
                    prev_es = es
                    es = []
                    for hl in range(2):
                        e_sb = es_pool.tile([128, 1024], BF16, tag="es",
                                            name=f"es{hl}")
                        nc.scalar.activation(e_sb[:], s_list[hl][:], AF.Exp,
                                             scale=SCALE2)
                        es.append(e_sb)
                    # AV for step j-1 (software-pipelined one step behind)
                    if j > 0:
                        for hl in range(2):
                            for n in range(2):
                                nc.tensor.matmul(
                                    o_h[hl][:, n * 512:(n + 1) * 512],
                                    v_sb[:, j - 1, 2 * m + hl, :],
                                    prev_es[hl][:, n * 512:(n + 1) * 512],
                                    start=(j == 1), stop=False,
                                )
                    # interleaved extra work fills the remaining exp window
                    for (eui, ej, fn, a) in extras:
                        if (eui, ej) == (ui, j):
                            fn(*a)
                    # deferred normalize of the previous unit: Pool-side
                    # reciprocal/broadcast chain early; the DVE mults at the
                    # very end of the unit — anything emitted between them
                    # would stall behind their wait on the Pool chain via
                    # coalesced DVE-counter semaphores
                    if pending is not None and j == 1:
                        reps = emit_norm_recip(pending)
                    if pending is not None and j == 13:
                        emit_norm_mult(pending, reps, 0)
                    if pending is not None and j == 15:
                        emit_norm_mult(pending, reps, 1)
                        pending = None
                # epilogue AV for j=15
                for hl in range(2):
                    for n in range(2):
                        nc.tensor.matmul(
                            o_h[hl][:, n * 512:(n + 1) * 512],
                            v_sb[:, 15, 2 * m + hl, :],
                            es[hl][:, n * 512:(n + 1) * 512],
                            start=False, stop=True,
                        )
                # free the o psum slots quickly; normalize later from copy
                pending = emit_ocp_copy(m, i0, o_h, hl1_act=(ui == 3))

            # ---- tail: last normalize + Wo for ih=1 ----------------------
            # pre-start the pair-m0 half of the first two ih=1 Wo chunks on
            # the po slots (keeps PE warm through the normalize chain); the
            # ps slots stay free: one hosts the PE-side denominator
            # broadcast. Emitted BEFORE the norm chain so coalesced waits
            # can't round up into it.
            pre = []
            for ct in range(4):
                pool, tg = (ps, "s") if ct < 2 else (po, "o")
                acc = pool.tile([128, 1024], F32, tag=tg, name=f"wo_pre{ct}")
                for n in range(2):
                    nc.tensor.matmul(
                        acc[:, n * 512:(n + 1) * 512],
                        wo_sb[:, 0, ct * 128:(ct + 1) * 128],
                        oT_sbs[1][0][:, n * 512:(n + 1) * 512],
                        start=True, stop=False,
                    )
                pre.append(acc)
            reps = emit_norm_recip(pending)
            emit_norm_mult(pending, reps, 0)
            emit_norm_mult(pending, reps, 1)
            pending = None
            for ct in range(4):
                acc = pre[ct]
                for n in range(2):
                    nc.tensor.matmul(
                        acc[:, n * 512:(n + 1) * 512],
                        wo_sb[:, 1, ct * 128:(ct + 1) * 128],
                        oT_sbs[1][1][:, n * 512:(n + 1) * 512],
                        start=False, stop=True,
                    )
                ost = ost_pool.tile([128, 1024], BF16, tag="ost", name="ost")
                if ct % 2:
                    with nc.allow_low_precision(reason="bf16 output"):
                        nc.vector.tensor_copy(ost[:], acc[:])
                else:
                    nc.scalar.copy(ost[:], acc[:])
                nc.sync.dma_start(
                    outT[ct * 128:(ct + 1) * 128, 1024:2048], ost[:])
            for ct in range(4, 8):
                emit_wo_chunk(1, ct, cast_act=(ct % 2 == 0))

            ost_pool.release()
            st_pool.release()
            nd_pool.release()
            np_pool.release()
            es_pool.release()

    nc.compile()
    return nc


def kernel(x, Wq, Wk, Wv, Wo, bo):
    bf16 = ml_dtypes.bfloat16
    x = np.asarray(x, dtype=np.float32)
    Wq = np.asarray(Wq, dtype=np.float32)
    Wk = np.asarray(Wk, dtype=np.float32)
    Wv = np.asarray(Wv, dtype=np.float32)
    Wo = np.asarray(Wo, dtype=np.float32)
    bo = np.asarray(bo, dtype=np.float32)

    if "nc" not in _cache:
        _cache["nc"] = _build()
    nc = _cache["nc"]

    def prearr(w, kt):
        # [kt*128, c] -> partition-major [128, kt*c] (contiguous per-row DMA)
        r, c = w.shape
        return np.ascontiguousarray(
            w.reshape(kt, 128, c).transpose(1, 0, 2).reshape(128, kt * c)
        ).astype(bf16)

    xTs = [np.ascontiguousarray(x[b].T).astype(bf16) for b in range(B)]
    in_maps = []
    for core in range(8):
        b, hg = divmod(core, 4)
        sl = slice(hg * HD, (hg + 1) * HD)
        in_maps.append({
            "xT": xTs[b],
            "wq": prearr(Wq[:, sl], KT),
            "wk": prearr(Wk[:, sl], KT),
            "wv": prearr(Wv[:, sl], KT),
            "wo": prearr(Wo[sl, :], 2),
        })
    _cache["in_maps"] = in_maps

    res = run_bass_kernel_spmd(nc, in_maps, core_ids=list(range(8)))
    out = np.empty((B, L, C), dtype=np.float32)
    for b in range(B):
        acc = res.results[4 * b]["outT"].astype(np.float32)
        for hg in range(1, 4):
            acc = acc + res.results[4 * b + hg]["outT"].astype(np.float32)
        out[b] = acc.T + bo
    return out


# revision 61
# speedup vs baseline: 1.0122x; 1.0122x over previous
"""Multi-head attention (B=2, L=2048, H=16, D=64) on 8 TRN2 NeuronCores.

Sharding: core = (batch b, head-group hg); 2 batches x 4 groups of 4 heads.
Per core, for its batch and its 4 heads (2 head-pairs m):
    Q^T/K^T = W^T x^T           (pair-d on partitions; head 2m at rows 0:64,
                                 head 2m+1 at rows 64:128)
    V       = x W_v             (j on partitions, + ones column for denom)
    S^T     = K^T.T Q^T         (j on partitions, i free)
    P'      = exp(S^T/8)        (un-normalized softmax numerator, bf16)
    O'^T    = [V|1].T P'        (row 64 = softmax denominator)
    O^T     = O'[0:64] / O'[64]
    out^T  += Wo_rows^T O^T     (partial over head-group rows of Wo)
Host sums the 4 partials per batch, transposes, adds bo.

v2 schedule (from perfetto/NTFF analysis of the 286us baseline):
  - everything bf16 (same 0.414ns/row PE rate as f32r at N>=512, but half
    the DMA bytes; psum accumulation stays f32).
  - HW model: matmul ~= 13ns + N*0.414ns; LDWEIGHTS ~= K*1.2ns hidden
    under the previous matmul's streaming; exp [128,1024] ~= 1115ns + a
    ~114ns semaphore wait on the ACT queue -> attention is ACT-paced at
    ~2.4us per j-step, with ~0.5us/step of PE slack.
  - the baseline spent 85us on a serial DMA+projection head and 46us on a
    serial Wo+output tail; v2 streams projections/V/Wo-half-0 into the
    attention loop's PE slack and starts attention after only ~3 proj
    chunks (~13us).
  - normalize: DVE reciprocal straight on the [1,1024] denominator row,
    gpsimd partition_broadcast, DVE multiply; head-odd rows reach
    oT partitions 64:128 via a gpsimd SBUF->SBUF dma (engine lanes can't
    shift partitions).
"""

import sys

try:
    import concourse.bass as bass  # noqa: F401
except ImportError:  # pragma: no cover - path fallback
    sys.path.insert(0, "/opt/trn_rl_repo")

import numpy as np
import ml_dtypes
import concourse.bass as bass
import concourse.mybir as mybir
import concourse.tile as tile
from concourse import bacc
from concourse.bass_utils import run_bass_kernel_spmd

F32 = mybir.dt.float32
F32R = mybir.dt.float32r
BF16 = mybir.dt.bfloat16
AF = mybir.ActivationFunctionType

B = 2
L = 2048          # sequence length
C = 1024          # model dim
H_LOC = 4         # heads per core
D = 64            # head dim
HD = H_LOC * D    # 256 = local head-group width
KT = C // 128     # 8 k-tiles over the model dim
SCALE2 = float(D) ** -0.5  # 1/8, applied once inside exp

_cache = {}


def _build():
    nc = bacc.Bacc("TRN2", target_bir_lowering=False, debug=False, num_devices=8)

    # weights arrive host-pre-transposed to partition-major [128, k, c]
    # layout so their DMA is one contiguous run per partition (the natural
    # [(k p), c] rearrange loads 512B-strided gathers)
    xT = nc.declare_dram_parameter("xT", [C, L], BF16, isOutput=False)
    wq = nc.declare_dram_parameter("wq", [128, KT * HD], BF16, isOutput=False)
    wk = nc.declare_dram_parameter("wk", [128, KT * HD], BF16, isOutput=False)
    wv = nc.declare_dram_parameter("wv", [128, KT * HD], BF16, isOutput=False)
    wo = nc.declare_dram_parameter("wo", [128, 2 * C], BF16, isOutput=False)
    outT = nc.declare_dram_parameter("outT", [C, L], BF16, isOutput=True)

    with tile.TileContext(nc) as tc:
        with tc.tile_pool(name="sb", bufs=1) as sb, \
             tc.tile_pool(name="ps", bufs=2, space="PSUM") as ps, \
             tc.tile_pool(name="po", bufs=2, space="PSUM") as po:

            # ---- input DMA (issue order == availability order) -----------
            wq_sb = sb.tile([128, KT, HD], BF16, tag="wq")
            wk_sb = sb.tile([128, KT, HD], BF16, tag="wk")
            wv_sb = sb.tile([128, KT, HD], BF16, tag="wv")
            wo_sb = sb.tile([128, 2, C], BF16, tag="wo")
            xT_sb = sb.tile([128, KT, L], BF16, tag="xT")
            # few DMA issues (each dma_start costs ~650ns on the in-order
            # Sync queue), granular enough that the first proj chunks can
            # start before the whole xT half has landed
            xTr = xT.rearrange("(k p) l -> p k l", p=128)
            nc.sync.dma_start(xT_sb[:, 0:2, 0:1024], xTr[:, 0:2, 0:1024])
            nc.sync.dma_start(wq_sb[:, :, :], wq.rearrange("p (k c) -> p k c", k=KT))
            nc.sync.dma_start(wk_sb[:, :, :], wk.rearrange("p (k c) -> p k c", k=KT))
            for k in range(2, KT, 2):
                nc.sync.dma_start(xT_sb[:, k:k + 2, 0:1024], xTr[:, k:k + 2, 0:1024])
            nc.sync.dma_start(wv_sb[:, :, :], wv.rearrange("p (k c) -> p k c", k=KT))
            for k in range(0, KT, 4):
                nc.sync.dma_start(xT_sb[:, k:k + 4, 1024:2048],
                                  xTr[:, k:k + 4, 1024:2048])
            nc.sync.dma_start(wo_sb[:, :, :], wo.rearrange("p (k c) -> p k c", k=2))

            # ---- persistent SBUF ----------------------------------------
            qT_sb = sb.tile([128, 2, L], BF16, tag="qT")
            kT_sb = sb.tile([128, 2, L], BF16, tag="kT")
            # per-(i-half, pair) attention output tiles: avoids tile-granular
            # false deps between Wo reads and normalize writes, and keeps the
            # normalize-mult dest AP flat (a [0:64, m, :] view runs 2x slower
            # on DVE than a flat [64, 1024] write)
            oT_sbs = [[sb.tile([128, 1024], BF16, tag=f"oT{ih}{m}",
                               name=f"oT{ih}{m}")
                       for m in range(2)]
                      for ih in range(2)]
            v_sb = sb.tile([128, 16, H_LOC, D + 1], BF16, tag="v")
            nc.vector.memset(v_sb[:, :, :, D:D + 1], 1.0)

            es_pool = tc.alloc_tile_pool(name="es_pool", bufs=8)
            np_pool = tc.alloc_tile_pool(name="np_pool", bufs=4)
            nd_pool = tc.alloc_tile_pool(name="nd_pool", bufs=4)
            st_pool = tc.alloc_tile_pool(name="st_pool", bufs=2)
            ost_pool = tc.alloc_tile_pool(name="ost_pool", bufs=4)

            # ---- emission helpers ---------------------------------------
            def emit_pchunk(w_sb, t_sb, m, n, on_act=False):
                # q/k projection chunk: [128 pair-d, 512 i|j] of qT/kT.
                # on_act: do the psum->bf16 copy on the Scalar engine — in
                # the PE-bound early units ACT has idle slots and every DVE
                # op is coupling fodder for coalesced j-loop waits.
                p = ps.tile([128, 1024], F32, tag="s", name="proj")
                acc = p[:, 0:512]
                for k in range(KT):
                    nc.tensor.matmul(
                        acc,
                        w_sb[:, k, m * 128:(m + 1) * 128],
                        xT_sb[:, k, n * 512:(n + 1) * 512],
                        start=(k == 0), stop=(k == KT - 1),
                    )
                dst = t_sb[:, m, n * 512:(n + 1) * 512]
                if on_act:
                    nc.scalar.copy(dst, acc)
                else:
                    with nc.allow_low_precision(reason="bf16 matmul input"):
                        nc.vector.tensor_copy(dst, acc)

            def emit_vchunk(it, on_act=False):
                # V projection for one j-tile: [128 j, 4 h, 64 d]
                p = ps.tile([128, 1024], F32, tag="s", name="vproj")
                acc = p[:, 0:HD]
                for k in range(KT):
                    nc.tensor.matmul(
                        acc,
                        xT_sb[:, k, it * 128:(it + 1) * 128],
                        wv_sb[:, k, :],
                        start=(k == 0), stop=(k == KT - 1),
                    )
                dst = v_sb[:, it, :, 0:D]
                src = acc.rearrange("p (h d) -> p h d", h=H_LOC)
                if on_act:
                    nc.scalar.copy(dst, src)
                else:
                    with nc.allow_low_precision(reason="bf16 matmul input"):
                        nc.vector.tensor_copy(dst, src)

            def emit_wo_chunk(ih, ct, cast_act=False):
                # [128 c-rows, 1024 i] output chunk; accumulate both pairs
                i0 = ih * 1024
                acc = ps.tile([128, 1024], F32, tag="s", name="wo_ps")
                for kk in range(2):
                    for n in range(2):
                        nc.tensor.matmul(
                            acc[:, n * 512:(n + 1) * 512],
                            wo_sb[:, kk, ct * 128:(ct + 1) * 128],
                            oT_sbs[ih][kk][:, n * 512:(n + 1) * 512],
                            start=(kk == 0), stop=(kk == 1),
                        )
                ost = ost_pool.tile([128, 1024], BF16, tag="ost", name="ost")
                if cast_act:
                    nc.scalar.copy(ost[:], acc[:])
                else:
                    with nc.allow_low_precision(reason="bf16 output"):
                        nc.vector.tensor_copy(ost[:], acc[:])
                nc.sync.dma_start(
                    outT[ct * 128:(ct + 1) * 128, i0:i0 + 1024], ost[:])

            def wo_mm(acc, ih, ct, kk, start, stop):
                for n in range(2):
                    nc.tensor.matmul(
                        acc[:, n * 512:(n + 1) * 512],
                        wo_sb[:, kk, ct * 128:(ct + 1) * 128],
                        oT_sbs[ih][kk][:, n * 512:(n + 1) * 512],
                        start=start, stop=stop,
                    )

            # deferred normalization state: (m, i0, o_cps)
            def emit_ocp_copy(m, i0, o_h, hl1_act=False):
                # hl1_act: in the tail ACT is idle — run the two copies on
                # different engines so they overlap
                o_cps = []
                for hl in range(2):
                    o_cp = np_pool.tile([65, 1024], F32, tag="o_cp",
                                        name=f"o_cp{hl}")
                    if hl and hl1_act:
                        nc.scalar.copy(o_cp[:], o_h[hl][:])
                    else:
                        nc.vector.tensor_copy(o_cp[:], o_h[hl][:])
                    o_cps.append(o_cp)
                return (m, i0, o_cps)

            def emit_norm_recip(pend):
                # reciprocal of the [1,1024] denominator row at full DVE
                # width via a [128,8] dma-reshape (single-partition DVE ops
                # run ~6.5us and clog the queue); everything except the
                # reciprocal itself runs on the otherwise-idle Pool engine
                # so DVE head-of-line waits can't stall the j-loop
                m, i0, o_cps = pend
                dsqs, outs = [], []
                for hl in range(2):
                    dsq = nd_pool.tile([128, 8], F32, tag=f"dsq{hl}")
                    nc.sync.dma_start(dsq[:], o_cps[hl][64:65, :])
                    dsqs.append(dsq)
                for hl in range(2):
                    nc.vector.reciprocal(dsqs[hl][:], dsqs[hl][:])
                for hl in range(2):
                    dinv = nd_pool.tile([1, 1024], F32, tag=f"dinv{hl}")
                    nc.sync.dma_start(dinv[:], dsqs[hl][:])
                    rep = nd_pool.tile([64, 1024], F32, tag=f"rep{hl}")
                    nc.gpsimd.partition_broadcast(rep[:], dinv[:])
                    outs.append(rep)
                return outs

            def emit_norm_mult(pend, reps, hl, via_stage=False):
                # hl0 writing the 128-partition oT tile directly costs ~2.5us
                # on DVE vs 1.2us into a 64-partition stage; in the tail the
                # chain is latency-critical so stage+HWDGE-dma both halves
                m, i0, o_cps = pend
                ih = i0 // 1024
                with nc.allow_low_precision(reason="bf16 attention output"):
                    if hl == 0 and not via_stage:
                        nc.vector.tensor_mul(
                            oT_sbs[ih][m][0:64, :],
                            o_cps[hl][0:64, :], reps[hl][:])
                    else:
                        stage = st_pool.tile([64, 1024], BF16, tag="stage")
                        nc.vector.tensor_mul(
                            stage[:], o_cps[hl][0:64, :], reps[hl][:])
                        # HWDGE, not gpsimd: PE reads of oT would otherwise
                        # pick up a coalesced full-gpsimd-queue drain dep
                        nc.sync.dma_start(
                            oT_sbs[ih][m][hl * 64:(hl + 1) * 64, :], stage[:])

            # ---- head: q/k (m=0, i|j 0:1024) as four k-interleaved chunks
            # across 4 psum accumulators (2 ps + 2 po slots, all free before
            # attention) so they complete right behind the xT DMA stream
            # instead of serializing chunk-by-chunk at mid p-state
            hq0 = ps.tile([128, 1024], F32, tag="s", name="hq0")
            hq1 = ps.tile([128, 1024], F32, tag="s", name="hq1")
            hk0 = po.tile([128, 1024], F32, tag="o", name="hk0")
            hk1 = po.tile([128, 1024], F32, tag="o", name="hk1")
            for k in range(KT):
                for acc, w_sb, n in ((hq0, wq_sb, 0), (hq1, wq_sb, 1),
                                     (hk0, wk_sb, 0), (hk1, wk_sb, 1)):
                    nc.tensor.matmul(
                        acc[:, 0:512],
                        w_sb[:, k, 0:128],
                        xT_sb[:, k, n * 512:(n + 1) * 512],
                        start=(k == 0), stop=(k == KT - 1),
                    )
            # copies split across ACT and DVE (both idle) to halve the
            # serial latency ahead of the first S matmul
            for acc, t_sb, n, act in ((hq0, qT_sb, 0, True), (hq1, qT_sb, 1, False),
                                      (hk0, kT_sb, 0, True), (hk1, kT_sb, 1, False)):
                dst = t_sb[:, 0, n * 512:(n + 1) * 512]
                if act:
                    nc.scalar.copy(dst, acc[:, 0:512])
                else:
                    with nc.allow_low_precision(reason="bf16 matmul input"):
                        nc.vector.tensor_copy(dst, acc[:, 0:512])

            # extra PE/normalize work interleaved into the j-loops.
            # item: (ui, j, thunk) emitted right before step j of unit ui.
            extras = []

            def at(ui, j, fn, *a):
                extras.append((ui, j, fn, a))

            # unit 0 carries: all of V and its own late K, unit 1's Q/K
            # (all psum->sbuf copies on ACT: it idles in this PE-bound span;
            # v0/v1 as step-0 extras so they don't delay the first S matmul)
            at(0, 0, emit_vchunk, 0, True)
            at(0, 0, emit_vchunk, 1, True)
            for t in range(2, 16):                          # v jt 2..15 JIT
                at(0, t - 1, emit_vchunk, t, True)
            at(0, 5, emit_pchunk, wk_sb, kT_sb, 0, 2, True)   # u0 j8..11
            at(0, 9, emit_pchunk, wk_sb, kT_sb, 0, 3, True)   # u0 j12..15
            at(0, 11, emit_pchunk, wq_sb, qT_sb, 1, 0, True)  # u1 i 0..511
            at(0, 13, emit_pchunk, wq_sb, qT_sb, 1, 1, True)  # u1 i 512..1023
            at(0, 15, emit_pchunk, wk_sb, kT_sb, 1, 0, True)  # u1 j0..3
            # unit 1 carries: rest of its own K, unit 2's Q
            at(1, 0, emit_pchunk, wk_sb, kT_sb, 1, 1, True)   # u1 j4..7
            at(1, 4, emit_pchunk, wk_sb, kT_sb, 1, 2, True)   # u1 j8..11
            at(1, 6, emit_pchunk, wk_sb, kT_sb, 1, 3, True)   # u1 j12..15
            at(1, 9, emit_pchunk, wq_sb, qT_sb, 0, 2)         # u2 i 0..511
            at(1, 11, emit_pchunk, wq_sb, qT_sb, 0, 3)        # u2 i 512..1023
            # unit 2 carries: unit 3's Q
            at(2, 5, emit_pchunk, wq_sb, qT_sb, 1, 2)       # u3
            at(2, 7, emit_pchunk, wq_sb, qT_sb, 1, 3)       # u3
            # unit 3 carries all of Wo ih=0: norm(u1) finishes emission at
            # u2 step 15, so emission here guarantees the oT0 read order
            for ct, st in enumerate((2, 4, 6, 8, 10, 12, 14, 15)):
                at(3, st, emit_wo_chunk, 0, ct)

            # ---- attention units ----------------------------------------
            units = [(ih, m) for ih in range(2) for m in range(2)]
            pending = None   # o_cp record awaiting normalize
            reps = None

            for ui, (ih, m) in enumerate(units):
                i0 = ih * 1024
                o_h = []
                for hl in range(2):
                    of = po.tile([128, 1024], F32, tag="o", name=f"o_ps{hl}")
                    o_h.append(of[0:65, :])
                es = [None, None]
                for j in range(16):
                    # S first so exp can fire ASAP; its psum slot was freed
                    # by exp(j-2)
                    s_list = []
                    for hl in range(2):
                        r0 = hl * 64
                        s_ps = ps.tile([128, 1024], F32, tag="s", name=f"s_ps{hl}")
                        for n in range(2):
                            nc.tensor.matmul(
                                s_ps[:, n * 512:(n + 1) * 512],
                                kT_sb[r0:r0 + 64, m, j * 128:(j + 1) * 128],
                                qT_sb[r0:r0 + 64, m,
                                      i0 + n * 512:i0 + (n + 1) * 512],
                                start=True, stop=True,
                            )
                        s_list.append(s_ps)
                    prev_es = es
                    es = []
                    for hl in range(2):
                        e_sb = es_pool.tile([128, 1024], BF16, tag="es",
                                            name=f"es{hl}")
                        nc.scalar.activation(e_sb[:], s_list[hl][:], AF.Exp,
                                             scale=SCALE2)
                        es.append(e_sb)
                    # AV for step j-1 (software-pipelined one step behind)
                    if j > 0:
                        for hl in range(2):
                            for n in range(2):
                                nc.tensor.matmul(
                                    o_h[hl][:, n * 512:(n + 1) * 512],
                                    v_sb[:, j - 1, 2 * m + hl, :],
                                    prev_es[hl][:, n * 512:(n + 1) * 512],
                                    start=(j == 1), stop=False,
                                )
                    # interleaved extra work fills the remaining exp window
                    for (eui, ej, fn, a) in extras:
                        if (eui, ej) == (ui, j):
                            fn(*a)
                    # deferred normalize of the previous unit: Pool-side
                    # reciprocal/broadcast chain early; the DVE mults at the
                    # very end of the unit — anything emitted between them
                    # would stall behind their wait on the Pool chain via
                    # coalesced DVE-counter semaphores
                    if pending is not None and j == 1:
                        reps = emit_norm_recip(pending)
                    if pending is not None and j == 13:
                        emit_norm_mult(pending, reps, 0)
                    if pending is not None and j == 15:
                        emit_norm_mult(pending, reps, 1)
                        pending = None
                # epilogue AV for j=15
                for hl in range(2):
                    for n in range(2):
                        nc.tensor.matmul(
                            o_h[hl][:, n * 512:(n + 1) * 512],
                            v_sb[:, 15, 2 * m + hl, :],
                            es[hl][:, n * 512:(n + 1) * 512],
                            start=False, stop=True,
                        )
                # free the o psum slots quickly; normalize later from copy
                pending = emit_ocp_copy(m, i0, o_h, hl1_act=(ui == 3))

            # ---- tail: last normalize + Wo for ih=1 ----------------------
            # pre-start the pair-m0 half of the first two ih=1 Wo chunks on
            # the po slots (keeps PE warm through the normalize chain); the
            # ps slots stay free: one hosts the PE-side denominator
            # broadcast. Emitted BEFORE the norm chain so coalesced waits
            # can't round up into it.
            pre = []
            for ct in range(4):
                pool, tg = (ps, "s") if ct < 2 else (po, "o")
                acc = pool.tile([128, 1024], F32, tag=tg, name=f"wo_pre{ct}")
                for n in range(2):
                    nc.tensor.matmul(
                        acc[:, n * 512:(n + 1) * 512],
                        wo_sb[:, 0, ct * 128:(ct + 1) * 128],
                        oT_sbs[1][0][:, n * 512:(n + 1) * 512],
                        start=True, stop=False,
                    )
                pre.append(acc)
            reps = emit_norm_recip(pending)
            emit_norm_mult(pending, reps, 0)
            emit_norm_mult(pending, reps, 1)
            pending = None
            for ct in range(4):
                acc = pre[ct]
                for n in range(2):
                    nc.tensor.matmul(
                        acc[:, n * 512:(n + 1) * 512],
                        wo_sb[:, 1, ct * 128:(ct + 1) * 128],
                        oT_sbs[1][1][:, n * 512:(n + 1) * 512],
                        start=False, stop=True,
                    )
                ost = ost_pool.tile([128, 1024], BF16, tag="ost", name="ost")
                if ct % 2:
                    with nc.allow_low_precision(reason="bf16 output"):
                        nc.vector.tensor_copy(ost[:], acc[:])
                else:
                    nc.scalar.copy(ost[:], acc[:])
                nc.sync.dma_start(
                    outT[ct * 128:(ct + 1) * 128, 1024:2048], ost[:])
            for ct in range(4, 8):
                emit_wo_chunk(1, ct, cast_act=(ct % 2 == 0))

            ost_pool.release()
            st_pool.release()
            nd_pool.release()
            np_pool.release()
            es_pool.release()

    nc.compile()
    return nc


def kernel(x, Wq, Wk, Wv, Wo, bo):
    bf16 = ml_dtypes.bfloat16
    x = np.asarray(x, dtype=np.float32)
    Wq = np.asarray(Wq, dtype=np.float32)
    Wk = np.asarray(Wk, dtype=np.float32)
    Wv = np.asarray(Wv, dtype=np.float32)
    Wo = np.asarray(Wo, dtype=np.float32)
    bo = np.asarray(bo, dtype=np.float32)

    if "nc" not in _cache:
        _cache["nc"] = _build()
    nc = _cache["nc"]

    def prearr(w, kt):
        # [kt*128, c] -> partition-major [128, kt*c] (contiguous per-row DMA)
        r, c = w.shape
        return np.ascontiguousarray(
            w.reshape(kt, 128, c).transpose(1, 0, 2).reshape(128, kt * c)
        ).astype(bf16)

    xTs = [np.ascontiguousarray(x[b].T).astype(bf16) for b in range(B)]
    in_maps = []
    for core in range(8):
        b, hg = divmod(core, 4)
        sl = slice(hg * HD, (hg + 1) * HD)
        in_maps.append({
            "xT": xTs[b],
            "wq": prearr(Wq[:, sl], KT),
            "wk": prearr(Wk[:, sl], KT),
            "wv": prearr(Wv[:, sl], KT),
            "wo": prearr(Wo[sl, :], 2),
        })
    _cache["in_maps"] = in_maps

    res = run_bass_kernel_spmd(nc, in_maps, core_ids=list(range(8)))
    out = np.empty((B, L, C), dtype=np.float32)
    for b in range(B):
        acc = res.results[4 * b]["outT"].astype(np.float32)
        for hg in range(1, 4):
            acc = acc + res.results[4 * b + hg]["outT"].astype(np.float32)
        out[b] = acc.T + bo
    return out
